# revision 2
# baseline (speedup 1.0000x reference)
"""RBF/KNN interpolation kernel for Trainium2 (8 NeuronCores, data parallel).

Same low-rank cosine-expansion algorithm as the baseline kernel,
restructured for instruction count and engine balance:

  - GpSimd never touches PSUM and does only SBUF-cheap work (v-scale,
    combo lo-splits, broadcast copies) plus DMA issue; its tensor ops on
    the old critical path (2.2us copies) are gone.
  - The ft row duplication (rows 64:128 = rows 0:64) is baked into the
    u-matmul coefficient matrix (cmat2 cols 64:128 = cols 0:64): engine
    cost of RR/Sin is free-dim bound, so the duplicated [128, 1024] tile
    costs the same as [64, 1024] and the T-multiply is one op per block.
  - Coords are shifted by -0.5 on-device during the hi/lo split; phases
    become multiples of 0.25 (exact in fp16) and |u| <= 3.5.
  - Range reduction: ACT t1 = u + MAGIC (round via fp32 magic number),
    DVE STT am = (t1 - MAGIC) - u in-place into psum, ACT Sin(-2pi*am).
  - The 16 reduction matmuls accumulate into ONE psum bank at per-block
    output rows (num row j, den row 16+j) via column-shifted slices of a
    single [128,144] redt_base: no nd_rows copies at all.
  - lhsg comes straight from the M psum via two cross-partition
    tensor_scalar ops (M1/M0 are symmetric: no transpose needed).
  - Finalize once for all 8192 targets: 2 gather DMAs to out-nat
    [128,128], reciprocal+multiply+x4 broadcast at full width, then 10
    contiguous 128KB sample writes from SBUF (no DRAM->DRAM traffic).
  - 3 input DMAs total (tcn / fbig / blob16), constants host-packed.
"""

import os
from contextlib import ExitStack
from functools import lru_cache

import numpy as np

import concourse.bass as bass
import concourse.bacc as bacc
import concourse.tile as tile
from concourse import mybir
from concourse.bass_utils import run_bass_kernel_spmd

F32 = mybir.dt.float32
F16 = mybir.dt.float16
U32 = mybir.dt.uint32
AF = mybir.ActivationFunctionType
ALU = mybir.AluOpType

B = 8
T_IN = 4
N_IN = 4096
V_IN = 3
N_OUT = 8192
S = 10
T_OUT = 4
SIG = 0.1
MM = 12                  # max cosine harmonic
TWO_PI = 2.0 * np.pi
MAGIC = 12582912.0       # 1.5 * 2^23: fp32 (x + MAGIC) - MAGIC == round(x)

IC = N_IN // 128         # 32 input chunks
TC = N_OUT // 128        # 64 target chunks
NSB = 8                  # superblocks of 1024 targets
NBLK = 16                # blocks of 512 targets

XCOS = list(range(0, 13))        # m = 0..12
XSIN = list(range(13, 25))       # m = 1..12
YCOS = list(range(25, 38))
YSIN = list(range(38, 50))


def _am(m):
    v = SIG * np.sqrt(2 * np.pi) * np.exp(-((np.pi * m * SIG) ** 2) / 2)
    return v / 2 if m == 0 else v


def _fold(t):
    # fold to [-0.5, 0.5); exact for multiples of 0.25
    return ((t + 0.5) % 1.0) - 0.5


def _consts():
    # cmat2 [8, 128]: u = (m/2)(x-1/2 | y-1/2) + phase' via component rows
    # 0=xh 1=yh 2=xl 3=yl 4=ones; cols 64:128 duplicate 0:64.
    cmat2 = np.zeros((8, 128), np.float16)
    for i, m in enumerate(range(0, MM + 1)):
        for s in (XCOS[i], YCOS[i]):
            r = 0 if s < 25 else 1
            cmat2[r, s] = m / 2.0
            cmat2[r + 2, s] = m / 2.0
            cmat2[4, s] = _fold(0.25 + m / 4.0)
    for i, m in enumerate(range(1, MM + 1)):
        for s in (XSIN[i], YSIN[i]):
            r = 0 if s < 25 else 1
            cmat2[r, s] = m / 2.0
            cmat2[r + 2, s] = m / 2.0
            cmat2[4, s] = _fold(m / 4.0)
    cmat2[:, 64:128] = cmat2[:, 0:64]

    # redt_base [128, 144]: col 15 = a_m at x-slots (num), col 31 = a_m at
    # 64+x-slots (den).  Block j uses lhsT = redt_base[:, 15-j : 143-j],
    # putting num at psum row j and den at row 16+j.
    redt = np.zeros((128, 144), np.float16)
    for i, m in enumerate(range(0, MM + 1)):
        redt[XCOS[i], 15] = _am(m)
        redt[64 + XCOS[i], 31] = _am(m)
    for i, m in enumerate(range(1, MM + 1)):
        redt[XSIN[i], 15] = _am(m)
        redt[64 + XSIN[i], 31] = _am(m)

    # agy128: a_m at y-slots q and at 64+q (per-partition scalars for the
    # two lhsg halves)
    agy = np.zeros((128,), np.float32)
    for i, m in enumerate(range(0, MM + 1)):
        agy[YCOS[i]] = _am(m)
        agy[64 + YCOS[i]] = _am(m)
    for i, m in enumerate(range(1, MM + 1)):
        agy[YSIN[i]] = _am(m)
        agy[64 + YSIN[i]] = _am(m)

    ident = np.eye(128, dtype=np.float16)

    blob16 = np.zeros((128, 400), np.float16)
    blob16[:, 0:144] = redt
    blob16[:, 144:272] = ident
    # cmat2 replicated at partition offsets {0,32,64,96}: matmul lhsT/rhs
    # must share a base partition with the [32u:32u+8] transpose slices
    for u in range(4):
        blob16[32 * u:32 * u + 8, 272:400] = cmat2
    return blob16, agy


def build_kernel(tcx, tcn_h, fbig_h, blob_h, out_h):
    nc = tcx.nc
    stage = int(os.environ.get("KNEW_STAGE", "7"))

    def bail(cpool_grep):
        nc.vector.memset(cpool_grep[:, :], 0.0)
        engs = [nc.sync, nc.gpsimd, nc.scalar]
        for s in range(S):
            engs[s % 3].dma_start(
                out=out_h[:][s].rearrange("o t -> (o t)").rearrange(
                    "(p j) -> p j", p=128),
                in_=cpool_grep[:, :])

    with ExitStack() as ctx:
        cpool = ctx.enter_context(tcx.tile_pool(name="const", bufs=1))
        tcn = cpool.tile([128, 128], F32)
        fbig = cpool.tile([128, 99], F32)
        blob = cpool.tile([128, 400], F16)
        combo_t = cpool.tile([128, 32 * TC], F16)
        combo_i = cpool.tile([128, 32 * IC], F16)
        # matmul operands must sit at base partition 0 on this HW, so each
        # 32-row group of the transposes gets copied down to its own [8, N]
        # rows tile (u = chunk mod 4)
        tri = [cpool.tile([8, 1024], F16, name=f"tri{u}") for u in range(4)]
        trt = [cpool.tile([8, 2048], F16, name=f"trt{u}") for u in range(4)]
        fin = cpool.tile([128, IC * 128], F16)
        lhsg = cpool.tile([64, 128], F16)
        wsrc = cpool.tile([128, 512], F16)
        dum = cpool.tile([128, 8], F16)
        ndsb = cpool.tile([64, 512], F32)
        gnd = cpool.tile([128, 128], F32)
        rden = cpool.tile([128, 64], F32)
        gint = cpool.tile([128, 64], F32)
        grep = cpool.tile([128, 256], F32)
        grp10 = cpool.tile([128, 256 * S], F32)

        redt_of = lambda j: blob[:, 15 - j:143 - j]
        ident_s = blob[:, 144:272]
        cmat2 = blob[0:8, 272:400]
        cmat_in = blob[0:8, 272:336]
        vcol = fbig[:, 64:96]
        agy = fbig[:, 96:97]
        mgc = fbig[:, 97:98]
        mhalf = fbig[:, 98:99]

        # ---- head DMAs + PE warm-up + ACT trig-table preload ----
        nc.sync.dma_start(out=tcn[:, :], in_=tcn_h[:])
        nc.sync.dma_start(out=fbig[:, :], in_=fbig_h[:])
        nc.gpsimd.dma_start(out=blob[:, :], in_=blob_h[:])

        nc.gpsimd.memset(wsrc[:, :].bitcast(U32), 0)
        warm_cm = tcx.tile_pool(name="warm", bufs=2, space="PSUM")
        warm_pool = warm_cm.__enter__()
        for _ in range(6):
            wps = warm_pool.tile([128, 512], F32, tag="w")
            nc.tensor.matmul(wps[:, :], wsrc[:, 0:128], wsrc[:, :],
                             start=True, stop=True)
        warm_cm.__exit__(None, None, None)
        # first ACT op is a Sin so only the trig table set is ever loaded
        nc.scalar.activation(dum[:, :], wsrc[:, 0:8], AF.Sin)

        if stage <= 0:
            bail(grep)
            return

        # ---- hi/lo split with -0.5 shift into stride-32 combo tiles ----
        # full memsets first: rows 5:32 feed the transposes/matmuls and
        # must be finite (uninitialized SBUF can hold NaN; NaN*0 = NaN)
        c3t = combo_t.rearrange("p (c r) -> p c r", r=32)
        c3i = combo_i.rearrange("p (c r) -> p c r", r=32)
        nc.gpsimd.memset(combo_t[:, :].bitcast(U32), 0)
        nc.gpsimd.memset(combo_i[:, :].bitcast(U32), 0)
        nc.scalar.activation(c3t[:, :, 0], tcn[:, 0:64], AF.Identity,
                             bias=mhalf[:, 0:1])
        nc.scalar.activation(c3t[:, :, 1], tcn[:, 64:128], AF.Identity,
                             bias=mhalf[:, 0:1])
        nc.vector.scalar_tensor_tensor(c3t[:, :, 2], tcn[:, 0:64], 0.5,
                                       c3t[:, :, 0], op0=ALU.subtract,
                                       op1=ALU.subtract)
        nc.vector.scalar_tensor_tensor(c3t[:, :, 3], tcn[:, 64:128], 0.5,
                                       c3t[:, :, 1], op0=ALU.subtract,
                                       op1=ALU.subtract)
        nc.vector.memset(c3t[:, :, 4], 1.0)
        nc.scalar.activation(c3i[:, :, 0], fbig[:, 0:32], AF.Identity,
                             bias=mhalf[:, 0:1])
        nc.scalar.activation(c3i[:, :, 1], fbig[:, 32:64], AF.Identity,
                             bias=mhalf[:, 0:1])
        nc.vector.scalar_tensor_tensor(c3i[:, :, 2], fbig[:, 0:32], 0.5,
                                       c3i[:, :, 0], op0=ALU.subtract,
                                       op1=ALU.subtract)
        nc.vector.scalar_tensor_tensor(c3i[:, :, 3], fbig[:, 32:64], 0.5,
                                       c3i[:, :, 1], op0=ALU.subtract,
                                       op1=ALU.subtract)
        nc.vector.memset(c3i[:, :, 4], 1.0)

        if stage <= 1:
            bail(grep)
            return

        psm_cm = tcx.tile_pool(name="psm", bufs=1, space="PSUM")
        psm_pool = psm_cm.__enter__()
        psin_cm = tcx.tile_pool(name="psin", bufs=2, space="PSUM")
        psin_pool = psin_cm.__enter__()
        pst_cm = tcx.tile_pool(name="pst", bufs=2, space="PSUM")
        pst_pool = pst_cm.__enter__()

        # ---- batched PE transposes: 8 x [128,128] per [128,1024] psum
        # tile, then 4 cross-partition copies land 32-row groups at base 0
        def transpose_group(combo, rows, half, engs):
            pst = pst_pool.tile([128, 1024], F16, tag="pst")
            for t in range(8):
                nc.tensor.transpose(
                    pst[:, 128 * t:128 * (t + 1)],
                    combo[:, 1024 * half + 128 * t:1024 * half
                          + 128 * (t + 1)],
                    ident_s)
            for u in range(4):
                engs[u](rows[u][0:8, 1024 * half:1024 * (half + 1)],
                        pst[32 * u:32 * u + 8, :])

        ev = [nc.vector.tensor_copy, nc.scalar.copy,
              nc.vector.tensor_copy, nc.scalar.copy]
        transpose_group(combo_i, tri, 0, ev)

        if stage <= 2:
            pst_cm.__exit__(None, None, None)
            psin_cm.__exit__(None, None, None)
            psm_cm.__exit__(None, None, None)
            bail(grep)
            return

        # ---- input features (nat layout) + M accumulation ----
        # emission interleave keeps PE dense: u-matmuls of group g+1 sit
        # between M(g-1) and M(g) so M's vscale wait does not stall them
        psm = psm_pool.tile([128, 64], F32, tag="psm")
        fin3 = fin.rearrange("p (c w) -> p c w", w=128)
        t1i = [cpool.tile([128, 512], F32, name=f"t1i{g}") for g in range(4)]
        psins = [None] * 4

        def emit_input_u(g):
            psin = psin_pool.tile([128, 512], F32, tag="psin")
            for j in range(8):
                c = 8 * g + j
                lt = tri[c % 4][0:8, 128 * (c // 4):128 * (c // 4) + 128]
                nc.tensor.matmul(psin[:, 64 * j:64 * (j + 1)], lt,
                                 cmat_in, start=True, stop=True)
            psins[g] = psin

        def emit_input_rest(g):
            psin = psins[g]
            nc.scalar.activation(t1i[g][:, :], psin[:, :], AF.Identity,
                                 bias=mgc[:, 0:1])
            nc.vector.scalar_tensor_tensor(psin[:, :], t1i[g][:, :],
                                           MAGIC, psin[:, :],
                                           op0=ALU.subtract,
                                           op1=ALU.subtract)
            nc.scalar.activation(fin3[:, 8 * g:8 * (g + 1), 64:128],
                                 psin[:, :], AF.Sin, scale=-TWO_PI)
            for j in range(8):
                c = 8 * g + j
                nc.vector.tensor_scalar(fin3[:, c, 0:64],
                                        fin3[:, c, 64:128],
                                        vcol[:, c:c + 1], None,
                                        op0=ALU.mult)

        def emit_m(g):
            for j in range(8):
                c = 8 * g + j
                nc.tensor.matmul(psm[:, :], fin[:, 128 * c:128 * (c + 1)],
                                 fin[:, 128 * c + 64:128 * (c + 1)],
                                 start=(c == 0), stop=(c == IC - 1))

        emit_input_u(0)
        emit_input_rest(0)
        for g in range(4):
            if g + 1 < 4:
                emit_input_u(g + 1)
                emit_input_rest(g + 1)
            emit_m(g)
        if stage <= 3:
            pst_cm.__exit__(None, None, None)
            psin_cm.__exit__(None, None, None)
            psm_cm.__exit__(None, None, None)
            bail(grep)
            return

        # lhsg[q, 0:64] = a_q * M1[q, :],  lhsg[q, 64:128] = a_q * M0[q, :]
        # (M1/M0 symmetric; second op reads partitions 64:128)
        nc.vector.tensor_scalar(lhsg[0:64, 0:64], psm[0:64, 0:64],
                                agy[0:64, 0:1], None, op0=ALU.mult)
        nc.vector.tensor_scalar(lhsg[0:64, 64:128], psm[64:128, 0:64],
                                agy[64:128, 0:1], None, op0=ALU.mult)

        # ---- target transposes ----
        evt = [nc.vector.tensor_copy, nc.scalar.copy,
               nc.vector.tensor_copy, nc.scalar.copy]
        transpose_group(combo_t, trt, 0, evt)
        transpose_group(combo_t, trt, 1, evt[1:] + evt[:1])
        pst_cm.__exit__(None, None, None)
        psin_cm.__exit__(None, None, None)
        psm_cm.__exit__(None, None, None)

        if stage <= 5:
            bail(grep)
            return

        # ---- target stream (software-pipelined by 2 superblocks) ----
        with (
            tcx.tile_pool(name="psa", bufs=2, space="PSUM") as psa_pool,
            tcx.tile_pool(name="psg", bufs=1, space="PSUM") as psg_pool,
            tcx.tile_pool(name="psr", bufs=1, space="PSUM") as psr_pool,
            tcx.tile_pool(name="ftp", bufs=4) as ft_pool,
            tcx.tile_pool(name="ttp", bufs=3) as tt_pool,
            tcx.tile_pool(name="t1p", bufs=3) as t1_pool,
        ):
            psrs = [psr_pool.tile([128, 512], F32, name=f"psr{h}", tag=f"psr{h}")
                    for h in range(2)]
            fts = [None] * NSB

            def emit_features(sb):
                # host packs target chunks permuted so that trt[u] cols
                # [256 sb, 256(sb+1)) hold targets [1024 sb + 256 u, +256)
                # in natural order: 4 n=256 matmuls per superblock
                psa = psa_pool.tile([128, 1024], F32, tag="psa")
                for u in range(4):
                    nc.tensor.matmul(psa[:, 256 * u:256 * (u + 1)],
                                     cmat2,
                                     trt[u][0:8, 256 * sb:256 * (sb + 1)],
                                     start=True, stop=True)
                t1 = t1_pool.tile([128, 1024], F32, tag="t1")
                nc.scalar.activation(t1[:, :], psa[:, :], AF.Identity,
                                     bias=mgc[:, 0:1])
                nc.vector.scalar_tensor_tensor(psa[:, :], t1[:, :], MAGIC,
                                               psa[:, :], op0=ALU.subtract,
                                               op1=ALU.subtract)
                ft = ft_pool.tile([128, 1024], F16, tag="ft")
                nc.scalar.activation(ft[:, :], psa[:, :], AF.Sin,
                                     scale=-TWO_PI)
                fts[sb] = ft

            def finalize(H):
                # half H: blocks 8H..8H+7 -> psrs[H] num rows 8H:8H+8,
                # den rows 16+8H:24+8H; out-nat partitions 64H:64H+64
                p0 = 64 * H
                nb = 32 * H
                nc.scalar.copy(ndsb[nb:nb + 32, :], psrs[H][0:32, :])
                eng_den = nc.gpsimd if H == 0 else nc.scalar
                nc.sync.dma_start(
                    out=gnd[p0:p0 + 64, 0:64],
                    in_=ndsb[nb + 8 * H:nb + 8 * H + 8, :].rearrange(
                        "r (q k) -> r q k", k=64))
                eng_den.dma_start(
                    out=gnd[p0:p0 + 64, 64:128],
                    in_=ndsb[nb + 16 + 8 * H:nb + 24 + 8 * H, :].rearrange(
                        "r (q k) -> r q k", k=64))
                nc.vector.reciprocal(rden[p0:p0 + 64, :],
                                     gnd[p0:p0 + 64, 64:128])
                nc.vector.tensor_mul(gint[p0:p0 + 64, :],
                                     gnd[p0:p0 + 64, 0:64],
                                     rden[p0:p0 + 64, :])
                g4 = grep.rearrange("p (k t) -> p k t", t=4)
                nc.vector.tensor_copy(g4[p0:p0 + 64, :, 0],
                                      gint[p0:p0 + 64, :])
                nc.vector.tensor_copy(g4[p0:p0 + 64, :, 1],
                                      gint[p0:p0 + 64, :])
                nc.vector.tensor_copy(g4[p0:p0 + 64, :, 2],
                                      gint[p0:p0 + 64, :])
                nc.vector.tensor_copy(g4[p0:p0 + 64, :, 3],
                                      gint[p0:p0 + 64, :])
                engs = [nc.sync, nc.gpsimd] if H == 0 else [nc.sync, nc.scalar]
                for s in range(S):
                    engs[s % 2].dma_start(
                        out=out_h[:][s].rearrange(
                            "o t -> (o t)").rearrange(
                            "(p j) -> p j", p=128)[p0:p0 + 64, :],
                        in_=grep[p0:p0 + 64, :])

            emit_features(0)
            emit_features(1)
            for sb in range(NSB):
                # PE order G(sb) -> u(sb+2) -> red(sb): the u-matmuls fill
                # the PE while the DVE computes tt(sb)
                ft = fts[sb]
                psg = psg_pool.tile([128, 1024], F32, tag="psg")
                for h in range(2):
                    nc.tensor.matmul(psg[:, 512 * h:512 * (h + 1)],
                                     lhsg[0:64, :],
                                     ft[0:64, 512 * h:512 * (h + 1)],
                                     start=True, stop=True)
                tt = tt_pool.tile([128, 1024], F16, tag="tt")
                nc.vector.tensor_mul(tt[:, :], ft[:, :], psg[:, :])
                if sb + 2 < NSB:
                    emit_features(sb + 2)
                H = (2 * sb) // 8
                if os.environ.get("KNEW_FILL") and 0 < sb % 4:
                    # zero-adding filler keeps the PE busy through the
                    # tt wait (HAM only unthrottles under sustained work)
                    nc.tensor.matmul(psrs[H][:, :], wsrc[:, 0:128],
                                     wsrc[:, :], start=False, stop=False)
                for h in range(2):
                    j = 2 * sb + h
                    nc.tensor.matmul(psrs[H][:, :], redt_of(j),
                                     tt[:, 512 * h:512 * (h + 1)],
                                     start=(j % 8 == 0), stop=(j % 8 == 7))
                if sb == 3:
                    if stage <= 6:
                        bail(grep)
                        return
                    finalize(0)
            finalize(1)


@lru_cache(maxsize=2)
def build_nc():
    nc = bacc.Bacc("TRN2", target_bir_lowering=False, debug=False)
    tcn_h = nc.dram_tensor("tcn", [128, 128], F32, kind="ExternalInput")
    fbig_h = nc.dram_tensor("fbig", [128, 99], F32, kind="ExternalInput")
    blob_h = nc.dram_tensor("blob16", [128, 400], F16, kind="ExternalInput")
    out_h = nc.dram_tensor("out", [S, N_OUT, T_OUT], F32, kind="ExternalOutput")
    with tile.TileContext(nc) as tcx:
        build_kernel(tcx, tcn_h, fbig_h, blob_h, out_h)
    nc.compile()
    return nc


def _nat(a, chunks):
    # [N] -> [128, chunks] with nat[p, c] = a[c*128 + p]
    return np.ascontiguousarray(a.reshape(chunks, 128).T)


# target chunk permutation: combo position 8s+q holds chunk 8s+2(q%4)+q//4,
# so each trt[u] stores superblock-contiguous [8, 256] matmul operands
_TPERM = np.array([8 * (p // 8) + 2 * (p % 4) + (p % 8) // 4
                   for p in range(TC)])


def _natp(a):
    return _nat(a, TC)[:, _TPERM]


def _run(input_data, input_coords, target_coords, n_samples, trace=False):
    n_samples = int(n_samples)
    assert n_samples == S, f"kernel compiled for n_samples={S}, got {n_samples}"
    assert input_data.shape == (B, T_IN, N_IN, V_IN)
    nc = build_nc()
    blob16, agy = _consts()
    in_maps = []
    for b in range(B):
        tc = np.asarray(target_coords[b], dtype=np.float32)
        ic = np.asarray(input_coords[b], dtype=np.float32)
        v = np.asarray(input_data[b, T_IN - 1, :, 0], dtype=np.float32)
        fbig = np.empty((128, 99), np.float32)
        fbig[:, 0:32] = _nat(ic[:, 0], IC)
        fbig[:, 32:64] = _nat(ic[:, 1], IC)
        fbig[:, 64:96] = _nat(v, IC)
        fbig[:, 96] = agy
        fbig[:, 97] = MAGIC
        fbig[:, 98] = -0.5
        in_maps.append({
            "tcn": np.hstack([_natp(tc[:, 0]), _natp(tc[:, 1])]),
            "fbig": fbig,
            "blob16": blob16,
        })
    res = run_bass_kernel_spmd(nc, in_maps, list(range(B)), trace=trace)
    out = np.stack([res.results[b]["out"] for b in range(B)], axis=0)
    return out, res


def kernel(input_data, input_coords, target_coords, n_samples):
    out, _ = _run(
        np.asarray(input_data),
        np.asarray(input_coords),
        np.asarray(target_coords),
        n_samples,
    )
    return out


# revision 6
# speedup vs baseline: 1.1653x; 1.1653x over previous
"""RBF/KNN interpolation kernel for Trainium2 (8 NeuronCores, data parallel).

Same low-rank cosine-expansion algorithm as the baseline kernel,
restructured for instruction count and engine balance:

  - GpSimd never touches PSUM and does only SBUF-cheap work (v-scale,
    combo lo-splits, broadcast copies) plus DMA issue; its tensor ops on
    the old critical path (2.2us copies) are gone.
  - The ft row duplication (rows 64:128 = rows 0:64) is baked into the
    u-matmul coefficient matrix (cmat2 cols 64:128 = cols 0:64): engine
    cost of RR/Sin is free-dim bound, so the duplicated [128, 1024] tile
    costs the same as [64, 1024] and the T-multiply is one op per block.
  - Coords are shifted by -0.5 on-device during the hi/lo split; phases
    become multiples of 0.25 (exact in fp16) and |u| <= 3.5.
  - Range reduction: ACT t1 = u + MAGIC (round via fp32 magic number),
    DVE STT am = (t1 - MAGIC) - u in-place into psum, ACT Sin(-2pi*am).
  - The 16 reduction matmuls accumulate into ONE psum bank at per-block
    output rows (num row j, den row 16+j) via column-shifted slices of a
    single [128,144] redt_base: no nd_rows copies at all.
  - lhsg comes straight from the M psum via two cross-partition
    tensor_scalar ops (M1/M0 are symmetric: no transpose needed).
  - Finalize once for all 8192 targets: 2 gather DMAs to out-nat
    [128,128], reciprocal+multiply+x4 broadcast at full width, then 10
    contiguous 128KB sample writes from SBUF (no DRAM->DRAM traffic).
  - 3 input DMAs total (tcn / fbig / blob16), constants host-packed.
"""

import os
from contextlib import ExitStack
from functools import lru_cache

import numpy as np

import concourse.bass as bass
import concourse.bacc as bacc
import concourse.tile as tile
from concourse import mybir
from concourse.bass_utils import run_bass_kernel_spmd

F32 = mybir.dt.float32
F16 = mybir.dt.float16
U32 = mybir.dt.uint32
AF = mybir.ActivationFunctionType
ALU = mybir.AluOpType

B = 8
T_IN = 4
N_IN = 4096
V_IN = 3
N_OUT = 8192
S = 10
T_OUT = 4
SIG = 0.1
MM = 12                  # max cosine harmonic
TWO_PI = 2.0 * np.pi
MAGIC = 12582912.0       # 1.5 * 2^23: fp32 (x + MAGIC) - MAGIC == round(x)

IC = N_IN // 128         # 32 input chunks
TC = N_OUT // 128        # 64 target chunks
NSB = 8                  # superblocks of 1024 targets
NBLK = 16                # blocks of 512 targets

XCOS = list(range(0, 13))        # m = 0..12
XSIN = list(range(13, 25))       # m = 1..12
YCOS = list(range(25, 38))
YSIN = list(range(38, 50))


def _am(m):
    v = SIG * np.sqrt(2 * np.pi) * np.exp(-((np.pi * m * SIG) ** 2) / 2)
    return v / 2 if m == 0 else v


def _fold(t):
    # fold to [-0.5, 0.5); exact for multiples of 0.25
    return ((t + 0.5) % 1.0) - 0.5


def _consts():
    # cmat2 [8, 128]: u = (m/2)(x-1/2 | y-1/2) + phase' via component rows
    # 0=xh 1=yh 2=xl 3=yl 4=ones; cols 64:128 duplicate 0:64.
    cmat2 = np.zeros((8, 128), np.float16)
    for i, m in enumerate(range(0, MM + 1)):
        for s in (XCOS[i], YCOS[i]):
            r = 0 if s < 25 else 1
            cmat2[r, s] = m / 2.0
            cmat2[r + 2, s] = m / 2.0
            cmat2[4, s] = _fold(0.25 + m / 4.0)
    for i, m in enumerate(range(1, MM + 1)):
        for s in (XSIN[i], YSIN[i]):
            r = 0 if s < 25 else 1
            cmat2[r, s] = m / 2.0
            cmat2[r + 2, s] = m / 2.0
            cmat2[4, s] = _fold(m / 4.0)
    cmat2[:, 64:128] = cmat2[:, 0:64]

    # redt_base [128, 144]: col 15 = a_m at x-slots (num), col 31 = a_m at
    # 64+x-slots (den).  Block j uses lhsT = redt_base[:, 15-j : 143-j],
    # putting num at psum row j and den at row 16+j.
    redt = np.zeros((128, 144), np.float16)
    for i, m in enumerate(range(0, MM + 1)):
        redt[XCOS[i], 15] = _am(m)
        redt[64 + XCOS[i], 31] = _am(m)
    for i, m in enumerate(range(1, MM + 1)):
        redt[XSIN[i], 15] = _am(m)
        redt[64 + XSIN[i], 31] = _am(m)

    # agy128: a_m at y-slots q and at 64+q (per-partition scalars for the
    # two lhsg halves)
    agy = np.zeros((128,), np.float32)
    for i, m in enumerate(range(0, MM + 1)):
        agy[YCOS[i]] = _am(m)
        agy[64 + YCOS[i]] = _am(m)
    for i, m in enumerate(range(1, MM + 1)):
        agy[YSIN[i]] = _am(m)
        agy[64 + YSIN[i]] = _am(m)

    ident = np.eye(128, dtype=np.float16)

    blob16 = np.zeros((128, 400), np.float16)
    blob16[:, 0:144] = redt
    blob16[:, 144:272] = ident
    # cmat2 replicated at partition offsets {0,32,64,96}: matmul lhsT/rhs
    # must share a base partition with the [32u:32u+8] transpose slices
    for u in range(4):
        blob16[32 * u:32 * u + 8, 272:400] = cmat2
    return blob16, agy


def build_kernel(tcx, tcn_h, fbig_h, blob_h, out_h):
    nc = tcx.nc
    stage = int(os.environ.get("KNEW_STAGE", "7"))

    def bail(cpool_grep):
        nc.vector.memset(cpool_grep[:, :], 0.0)
        engs = [nc.sync, nc.gpsimd, nc.scalar]
        for s in range(S):
            engs[s % 3].dma_start(
                out=out_h[:][s].rearrange("o t -> (o t)").rearrange(
                    "(p j) -> p j", p=128),
                in_=cpool_grep[:, :])

    with ExitStack() as ctx:
        cpool = ctx.enter_context(tcx.tile_pool(name="const", bufs=1))
        tcn = cpool.tile([128, 128], F32)
        fbig = cpool.tile([128, 99], F32)
        blob = cpool.tile([128, 400], F16)
        combo_t = cpool.tile([128, 32 * TC], F16)
        combo_i = cpool.tile([128, 32 * IC], F16)
        # matmul operands must sit at base partition 0 on this HW, so each
        # 32-row group of the transposes gets copied down to its own [8, N]
        # rows tile (u = chunk mod 4)
        tri = [cpool.tile([8, 1024], F16, name=f"tri{u}") for u in range(4)]
        trt = [cpool.tile([8, 2048], F16, name=f"trt{u}") for u in range(4)]
        fin = cpool.tile([128, IC * 128], F16)
        lhsg = cpool.tile([64, 128], F16)
        wsrc = cpool.tile([128, 512], F16)
        dum = cpool.tile([128, 8], F16)
        ndsb = cpool.tile([64, 512], F32)
        gnd = cpool.tile([128, 128], F32)
        rden = cpool.tile([128, 64], F32)
        gint = cpool.tile([128, 64], F32)
        grep = cpool.tile([128, 256], F32)
        grp10 = cpool.tile([128, 256 * S], F32)

        redt_of = lambda j: blob[:, 15 - j:143 - j]
        ident_s = blob[:, 144:272]
        cmat2 = blob[0:8, 272:400]
        cmat_in = blob[0:8, 272:336]
        vcol = fbig[:, 64:96]
        agy = fbig[:, 96:97]
        mgc = fbig[:, 97:98]
        mhalf = fbig[:, 98:99]

        # ---- head DMAs + PE warm-up + ACT trig-table preload ----
        nc.sync.dma_start(out=tcn[:, :], in_=tcn_h[:])
        nc.scalar.dma_start(out=fbig[:, :], in_=fbig_h[:])
        nc.gpsimd.dma_start(out=blob[:, :], in_=blob_h[:])

        nc.gpsimd.memset(wsrc[:, :].bitcast(U32), 0)
        warm_cm = tcx.tile_pool(name="warm", bufs=2, space="PSUM")
        warm_pool = warm_cm.__enter__()
        for _ in range(12):
            wps = warm_pool.tile([128, 512], F32, tag="w")
            nc.tensor.matmul(wps[:, :], wsrc[:, 0:128], wsrc[:, :],
                             start=True, stop=True)
        warm_cm.__exit__(None, None, None)
        # first ACT op is a Sin so only the trig table set is ever loaded
        nc.scalar.activation(dum[:, :], wsrc[:, 0:8], AF.Sin)

        if stage <= 0:
            bail(grep)
            return

        # ---- hi/lo split with -0.5 shift into stride-32 combo tiles ----
        # full memsets first: rows 5:32 feed the transposes/matmuls and
        # must be finite (uninitialized SBUF can hold NaN; NaN*0 = NaN)
        c3t = combo_t.rearrange("p (c r) -> p c r", r=32)
        c3i = combo_i.rearrange("p (c r) -> p c r", r=32)
        nc.gpsimd.memset(combo_t[:, :].bitcast(U32), 0)
        nc.gpsimd.memset(combo_i[:, :].bitcast(U32), 0)
        nc.scalar.activation(c3t[:, :, 0], tcn[:, 0:64], AF.Identity,
                             bias=mhalf[:, 0:1])
        nc.scalar.activation(c3t[:, :, 1], tcn[:, 64:128], AF.Identity,
                             bias=mhalf[:, 0:1])
        nc.vector.scalar_tensor_tensor(c3t[:, :, 2], tcn[:, 0:64], 0.5,
                                       c3t[:, :, 0], op0=ALU.subtract,
                                       op1=ALU.subtract)
        nc.vector.scalar_tensor_tensor(c3t[:, :, 3], tcn[:, 64:128], 0.5,
                                       c3t[:, :, 1], op0=ALU.subtract,
                                       op1=ALU.subtract)
        nc.vector.memset(c3t[:, :, 4], 1.0)
        nc.scalar.activation(c3i[:, :, 0], fbig[:, 0:32], AF.Identity,
                             bias=mhalf[:, 0:1])
        nc.scalar.activation(c3i[:, :, 1], fbig[:, 32:64], AF.Identity,
                             bias=mhalf[:, 0:1])
        nc.vector.scalar_tensor_tensor(c3i[:, :, 2], fbig[:, 0:32], 0.5,
                                       c3i[:, :, 0], op0=ALU.subtract,
                                       op1=ALU.subtract)
        nc.vector.scalar_tensor_tensor(c3i[:, :, 3], fbig[:, 32:64], 0.5,
                                       c3i[:, :, 1], op0=ALU.subtract,
                                       op1=ALU.subtract)
        nc.vector.memset(c3i[:, :, 4], 1.0)

        if stage <= 1:
            bail(grep)
            return

        psm_cm = tcx.tile_pool(name="psm", bufs=1, space="PSUM")
        psm_pool = psm_cm.__enter__()
        psin_cm = tcx.tile_pool(name="psin", bufs=2, space="PSUM")
        psin_pool = psin_cm.__enter__()
        pst_cm = tcx.tile_pool(name="pst", bufs=2, space="PSUM")
        pst_pool = pst_cm.__enter__()

        # ---- batched PE transposes: 8 x [128,128] per [128,1024] psum
        # tile, then 4 cross-partition copies land 32-row groups at base 0
        def transpose_group(combo, rows, half, engs):
            pst = pst_pool.tile([128, 1024], F16, tag="pst")
            for t in range(8):
                nc.tensor.transpose(
                    pst[:, 128 * t:128 * (t + 1)],
                    combo[:, 1024 * half + 128 * t:1024 * half
                          + 128 * (t + 1)],
                    ident_s)
            for u in range(4):
                engs[u](rows[u][0:8, 1024 * half:1024 * (half + 1)],
                        pst[32 * u:32 * u + 8, :])

        ev = [nc.vector.tensor_copy, nc.scalar.copy,
              nc.vector.tensor_copy, nc.scalar.copy]
        transpose_group(combo_i, tri, 0, ev)

        if stage <= 2:
            pst_cm.__exit__(None, None, None)
            psin_cm.__exit__(None, None, None)
            psm_cm.__exit__(None, None, None)
            bail(grep)
            return

        # ---- input features (nat layout) + M accumulation ----
        # emission interleave keeps PE dense: u-matmuls of group g+1 sit
        # between M(g-1) and M(g) so M's vscale wait does not stall them
        psm = psm_pool.tile([128, 64], F32, tag="psm")
        fin3 = fin.rearrange("p (c w) -> p c w", w=128)
        t1i = [cpool.tile([128, 512], F32, name=f"t1i{g}") for g in range(4)]
        psins = [None] * 4

        def emit_input_u(g):
            psin = psin_pool.tile([128, 512], F32, tag="psin")
            for j in range(8):
                c = 8 * g + j
                lt = tri[c % 4][0:8, 128 * (c // 4):128 * (c // 4) + 128]
                nc.tensor.matmul(psin[:, 64 * j:64 * (j + 1)], lt,
                                 cmat_in, start=True, stop=True)
            psins[g] = psin

        def emit_input_rest(g):
            psin = psins[g]
            nc.scalar.activation(t1i[g][:, :], psin[:, :], AF.Identity,
                                 bias=mgc[:, 0:1])
            nc.vector.scalar_tensor_tensor(psin[:, :], t1i[g][:, :],
                                           MAGIC, psin[:, :],
                                           op0=ALU.subtract,
                                           op1=ALU.subtract)
            nc.scalar.activation(fin3[:, 8 * g:8 * (g + 1), 64:128],
                                 psin[:, :], AF.Sin, scale=-TWO_PI)
            for j in range(8):
                c = 8 * g + j
                nc.vector.tensor_scalar(fin3[:, c, 0:64],
                                        fin3[:, c, 64:128],
                                        vcol[:, c:c + 1], None,
                                        op0=ALU.mult)

        def emit_m(g):
            for j in range(8):
                c = 8 * g + j
                nc.tensor.matmul(psm[:, :], fin[:, 128 * c:128 * (c + 1)],
                                 fin[:, 128 * c + 64:128 * (c + 1)],
                                 start=(c == 0), stop=(c == IC - 1))

        emit_input_u(0)
        emit_input_rest(0)
        for g in range(4):
            if g + 1 < 4:
                emit_input_u(g + 1)
                emit_input_rest(g + 1)
            emit_m(g)
        if stage <= 3:
            pst_cm.__exit__(None, None, None)
            psin_cm.__exit__(None, None, None)
            psm_cm.__exit__(None, None, None)
            bail(grep)
            return

        # lhsg[q, 0:64] = a_q * M1[q, :],  lhsg[q, 64:128] = a_q * M0[q, :]
        # (M1/M0 symmetric; second op reads partitions 64:128)
        nc.vector.tensor_scalar(lhsg[0:64, 0:64], psm[0:64, 0:64],
                                agy[0:64, 0:1], None, op0=ALU.mult)
        nc.vector.tensor_scalar(lhsg[0:64, 64:128], psm[64:128, 0:64],
                                agy[64:128, 0:1], None, op0=ALU.mult)

        # ---- target transposes ----
        evt = [nc.vector.tensor_copy, nc.scalar.copy,
               nc.vector.tensor_copy, nc.scalar.copy]
        transpose_group(combo_t, trt, 0, evt)
        transpose_group(combo_t, trt, 1, evt[1:] + evt[:1])
        pst_cm.__exit__(None, None, None)
        psin_cm.__exit__(None, None, None)
        psm_cm.__exit__(None, None, None)

        if stage <= 5:
            bail(grep)
            return

        # ---- target stream (software-pipelined by 2 superblocks) ----
        with (
            tcx.tile_pool(name="psa", bufs=2, space="PSUM") as psa_pool,
            tcx.tile_pool(name="psg", bufs=1, space="PSUM") as psg_pool,
            tcx.tile_pool(name="psr", bufs=1, space="PSUM") as psr_pool,
            tcx.tile_pool(name="ftp", bufs=4) as ft_pool,
            tcx.tile_pool(name="ttp", bufs=3) as tt_pool,
            tcx.tile_pool(name="t1p", bufs=3) as t1_pool,
        ):
            psrs = [psr_pool.tile([128, 512], F32, name=f"psr{h}", tag=f"psr{h}")
                    for h in range(2)]
            fts = [None] * NSB

            def emit_features(sb):
                # host packs target chunks permuted so that trt[u] cols
                # [256 sb, 256(sb+1)) hold targets [1024 sb + 256 u, +256)
                # in natural order: 4 n=256 matmuls per superblock
                psa = psa_pool.tile([128, 1024], F32, tag="psa")
                for u in range(4):
                    nc.tensor.matmul(psa[:, 256 * u:256 * (u + 1)],
                                     cmat2,
                                     trt[u][0:8, 256 * sb:256 * (sb + 1)],
                                     start=True, stop=True)
                t1 = t1_pool.tile([128, 1024], F32, tag="t1")
                nc.scalar.activation(t1[:, :], psa[:, :], AF.Identity,
                                     bias=mgc[:, 0:1])
                nc.vector.scalar_tensor_tensor(psa[:, :], t1[:, :], MAGIC,
                                               psa[:, :], op0=ALU.subtract,
                                               op1=ALU.subtract)
                ft = ft_pool.tile([128, 1024], F16, tag="ft")
                nc.scalar.activation(ft[:, :], psa[:, :], AF.Sin,
                                     scale=-TWO_PI)
                fts[sb] = ft

            def finalize(H):
                # half H: blocks 8H..8H+7 -> psrs[H] num rows 8H:8H+8,
                # den rows 16+8H:24+8H; out-nat partitions 64H:64H+64
                p0 = 64 * H
                nb = 32 * H
                nc.scalar.copy(ndsb[nb:nb + 32, :], psrs[H][0:32, :])
                eng_den = nc.gpsimd if H == 0 else nc.scalar
                nc.sync.dma_start(
                    out=gnd[p0:p0 + 64, 0:64],
                    in_=ndsb[nb + 8 * H:nb + 8 * H + 8, :].rearrange(
                        "r (q k) -> r q k", k=64))
                eng_den.dma_start(
                    out=gnd[p0:p0 + 64, 64:128],
                    in_=ndsb[nb + 16 + 8 * H:nb + 24 + 8 * H, :].rearrange(
                        "r (q k) -> r q k", k=64))
                nc.vector.reciprocal(rden[p0:p0 + 64, :],
                                     gnd[p0:p0 + 64, 64:128])
                nc.vector.tensor_mul(gint[p0:p0 + 64, :],
                                     gnd[p0:p0 + 64, 0:64],
                                     rden[p0:p0 + 64, :])
                g4 = grep.rearrange("p (k t) -> p k t", t=4)
                nc.vector.tensor_copy(g4[p0:p0 + 64, :, 0],
                                      gint[p0:p0 + 64, :])
                nc.vector.tensor_copy(g4[p0:p0 + 64, :, 1],
                                      gint[p0:p0 + 64, :])
                nc.vector.tensor_copy(g4[p0:p0 + 64, :, 2],
                                      gint[p0:p0 + 64, :])
                nc.vector.tensor_copy(g4[p0:p0 + 64, :, 3],
                                      gint[p0:p0 + 64, :])
                engs = [nc.sync, nc.gpsimd] if H == 0 else [nc.sync, nc.scalar]
                for s in range(S):
                    engs[s % 2].dma_start(
                        out=out_h[:][s].rearrange(
                            "o t -> (o t)").rearrange(
                            "(p j) -> p j", p=128)[p0:p0 + 64, :],
                        in_=grep[p0:p0 + 64, :])

            emit_features(0)
            emit_features(1)
            for sb in range(NSB):
                # PE order G(sb) -> u(sb+2) -> red(sb): the u-matmuls fill
                # the PE while the DVE computes tt(sb)
                ft = fts[sb]
                psg = psg_pool.tile([128, 1024], F32, tag="psg")
                for h in range(2):
                    nc.tensor.matmul(psg[:, 512 * h:512 * (h + 1)],
                                     lhsg[0:64, :],
                                     ft[0:64, 512 * h:512 * (h + 1)],
                                     start=True, stop=True)
                tt = tt_pool.tile([128, 1024], F16, tag="tt")
                nc.vector.tensor_mul(tt[:, :], ft[:, :], psg[:, :])
                if sb + 2 < NSB:
                    emit_features(sb + 2)
                H = (2 * sb) // 8
                if os.environ.get("KNEW_FILL") and 0 < sb % 4:
                    # zero-adding filler keeps the PE busy through the
                    # tt wait (HAM only unthrottles under sustained work)
                    nc.tensor.matmul(psrs[H][:, :], wsrc[:, 0:128],
                                     wsrc[:, :], start=False, stop=False)
                for h in range(2):
                    j = 2 * sb + h
                    nc.tensor.matmul(psrs[H][:, :], redt_of(j),
                                     tt[:, 512 * h:512 * (h + 1)],
                                     start=(j % 8 == 0), stop=(j % 8 == 7))
                if sb == 3:
                    if stage <= 6:
                        bail(grep)
                        return
                    finalize(0)
            finalize(1)


@lru_cache(maxsize=2)
def build_nc():
    nc = bacc.Bacc("TRN2", target_bir_lowering=False, debug=False)
    tcn_h = nc.dram_tensor("tcn", [128, 128], F32, kind="ExternalInput")
    fbig_h = nc.dram_tensor("fbig", [128, 99], F32, kind="ExternalInput")
    blob_h = nc.dram_tensor("blob16", [128, 400], F16, kind="ExternalInput")
    out_h = nc.dram_tensor("out", [S, N_OUT, T_OUT], F32, kind="ExternalOutput")
    with tile.TileContext(nc) as tcx:
        build_kernel(tcx, tcn_h, fbig_h, blob_h, out_h)
    nc.compile()
    return nc


def _nat(a, chunks):
    # [N] -> [128, chunks] with nat[p, c] = a[c*128 + p]
    return np.ascontiguousarray(a.reshape(chunks, 128).T)


# target chunk permutation: combo position 8s+q holds chunk 8s+2(q%4)+q//4,
# so each trt[u] stores superblock-contiguous [8, 256] matmul operands
_TPERM = np.array([8 * (p // 8) + 2 * (p % 4) + (p % 8) // 4
                   for p in range(TC)])


def _natp(a):
    return _nat(a, TC)[:, _TPERM]


def _run(input_data, input_coords, target_coords, n_samples, trace=False):
    n_samples = int(n_samples)
    assert n_samples == S, f"kernel compiled for n_samples={S}, got {n_samples}"
    assert input_data.shape == (B, T_IN, N_IN, V_IN)
    nc = build_nc()
    blob16, agy = _consts()
    in_maps = []
    for b in range(B):
        tc = np.asarray(target_coords[b], dtype=np.float32)
        ic = np.asarray(input_coords[b], dtype=np.float32)
        v = np.asarray(input_data[b, T_IN - 1, :, 0], dtype=np.float32)
        fbig = np.empty((128, 99), np.float32)
        fbig[:, 0:32] = _nat(ic[:, 0], IC)
        fbig[:, 32:64] = _nat(ic[:, 1], IC)
        fbig[:, 64:96] = _nat(v, IC)
        fbig[:, 96] = agy
        fbig[:, 97] = MAGIC
        fbig[:, 98] = -0.5
        in_maps.append({
            "tcn": np.hstack([_natp(tc[:, 0]), _natp(tc[:, 1])]),
            "fbig": fbig,
            "blob16": blob16,
        })
    res = run_bass_kernel_spmd(nc, in_maps, list(range(B)), trace=trace)
    out = np.stack([res.results[b]["out"] for b in range(B)], axis=0)
    return out, res


def kernel(input_data, input_coords, target_coords, n_samples):
    out, _ = _run(
        np.asarray(input_data),
        np.asarray(input_coords),
        np.asarray(target_coords),
        n_samples,
    )
    return out
